# revision 75
# baseline (speedup 1.0000x reference)
"""Trainium2 Bass kernel for nn_Attentionv2 (B=8, N=1024, C=768, H=12, D=64).

Strategy: data-parallel over batch — one batch element per NeuronCore (8 cores).
Per core, multi-head attention is computed entirely in the "transposed"
orientation so no on-chip transposes are needed:

  QT[h*64+d, n] = sum_c WqT[c, h*64+d] * xT[c, n]     (head-pair tiles)
  KT likewise; V[n, h*64+d] = sum_c xT[c, n-tile] * WvT[c, :]
  ST[m, n]  = sum_d KT[d, m] * QT[d, n]               (scores transposed;
               the two heads of a pair sit on partitions 0-63 / 64-127 so
               their K=64 matmuls row-tile into the two PE array halves)
  ET        = exp(ST * 1/8)                            (no max-subtraction:
                                                        scores are O(1) here)
  PV lhsT   = [V_h | ones(64 cols)]  =>  out rows 0-63 = OT_h (unnorm),
               rows 64-127 = softmax denominator replicated 64x (free
               partition-broadcast done by the PE)
  OT_norm   = OT * (1/Z)                               (reciprocal via DVE,
                                                        straight from PSUM)
  y[n, o]   = sum_c OT_norm[c, n] * WpT[c, o] + bp[o]

Matmul operands are fp16 (full-rate PE, fast weight loads, HAM-warm clocks);
all accumulation is fp32 in PSUM.

Changes vs the original baseline:
  - q/k weights are pair-major on the host so each pair's slice is one
    contiguous DMA; all x/w input DMAs are fully contiguous per partition
    via the contraction-row remap c = 6p + o (both MM operands of every
    contraction use the same map, so the accumulated sum is unchanged).
  - QK(j+1) is emitted BEFORE PV(j-1) at each pair's mt==1 slot so the
    qt/kt evacuation copies land early in the DVE static order and the
    next pair's first scores are not scheduled behind the PV+normalize
    block (the exp chain otherwise stalls ~3.5us at pair boundaries).
  - the output projection is split: the pair 0-4 contribution for 6 of 8
    row tiles runs as PE filler inside the ACT-bound last-pair exp window
    (reusing the idle "qk" PSUM slots) with the bias folded into that
    evacuation add; the tail then only needs the pair-5 rank-128 update.
  - the tail update folds the phase-A partial back in via an identity
    matmul, so the final evacuation is a plain PSUM->SBUF copy that
    alternates between the (by then idle) ACT and DVE engines.
  - the last pair's PV goes through the freed ps_s banks for hh=1 so it
    overlaps hh=0's normalize instead of serializing on the 2-slot ps_o
    rotation, and its Z-row evacuations run on ACT.

  - the tail runs as two half-width passes with ONE psum tile per step
    (two tiles per step turn the 2-slot rotation into a serial ladder),
    evacuation copies alternate ACT/DVE, and the output DMA is split
    across the sync and gpsimd queues (a DMA trigger costs ~0.6us ON the
    issuing engine and would serialize with the ACT copies).

NOTE on scheduling: the Tile scheduler's static per-engine order is very
sensitive to emission order and its cost model drifts from real DMA/HAM
timing; many locally-reasonable reorderings (x-first DMA shapes, V tiles
inside pair 0, sim-time floor pinning of the whole pipeline) measured
WORSE on hardware.  This configuration is the empirical optimum of ~20
variants: this binary measures 184.4-185.0us on a healthy device (vs
198-200us baseline); identical binaries vary ~3us run-to-run plus a slow
upward drift over a session, and the device intermittently enters a
~220-234us throttled state — never trust a single-run A/B.  Pool-size
changes (ets, ys2 depth) improved monotonically; emission reshuffles were
a lottery — prefer the former.

The two highest-leverage fixes, in order: (1) ets pool at 32 bufs = full
double-buffering of the exp->PV tiles; at 24 the next pair's scores
exhaust the pool before PV(j) frees it, stalling the exp chain 2.5-4us at
EVERY pair boundary (ets=32 moved exp-chain-end from 170.7 to 157.6us);
(2) the x-thirds-first DMA order above (first exp 29us vs 32-37).
"""

import numpy as np

P = 128
B, N, C = 8, 1024, 768
H, D = 12, 64
SCALE = D ** -0.5  # 0.125
CT = C // P   # 6 contraction chunks
NT = N // P   # 8 sequence tiles
HP = H // 2   # 6 head pairs
NCORES = 8

_cache = {}


def _build_nc():
    import concourse.bass as bass
    import concourse.mybir as mybir
    import concourse.tile as tile
    from concourse import bacc

    f32 = mybir.dt.float32
    f16 = mybir.dt.float16
    Exp = mybir.ActivationFunctionType.Exp

    nc = bacc.Bacc("TRN2", target_bir_lowering=False, debug=False,
                   enable_asserts=False)

    xT = nc.dram_tensor("xT", [C, N], f16, kind="ExternalInput").ap()
    ident = nc.dram_tensor("ident", [P, P], f16, kind="ExternalInput").ap()
    wqT = nc.dram_tensor("wqT", [HP, C, 2 * D], f16, kind="ExternalInput").ap()
    wkT = nc.dram_tensor("wkT", [HP, C, 2 * D], f16, kind="ExternalInput").ap()
    wvT = nc.dram_tensor("wvT", [C, H * D], f16, kind="ExternalInput").ap()
    wpT = nc.dram_tensor("wpT", [C, C], f16, kind="ExternalInput").ap()
    bpb = nc.dram_tensor("bpb", [P, C], f32, kind="ExternalInput").ap()
    y = nc.dram_tensor("y", [N, C], f32, kind="ExternalOutput").ap()

    mm = nc.tensor.matmul

    # contraction-row remap: partition p of chunk o holds row c = 6p + o,
    # so every per-partition DMA run is contiguous in HBM.  x/wq/wk/wv all
    # use this map (both matmul operands see the same permutation); wp keeps
    # chunk==pair / partition==row-within-pair since its contraction runs
    # over the concat-head dim whose layout is fixed by ot.
    xTr = xT.rearrange("(p o) n -> p o n", o=CT)
    wqTr = wqT.rearrange("h (p o) f -> p h o f", o=CT)
    wkTr = wkT.rearrange("h (p o) f -> p h o f", o=CT)
    wvTr = wvT.rearrange("(p o) f -> p o f", o=CT)
    wpTr = wpT.rearrange("(o p) f -> p o f", p=P)

    with tile.TileContext(nc) as tc:
        with tc.tile_pool(name="persist", bufs=1) as persist:
            qt = persist.tile([P, HP, N], f16)        # QT: head pair j rows
            kt = persist.tile([P, HP, N], f16)
            vp = persist.tile([P, NT, H, 2 * D], f16)  # [Vh | ones]
            ot = persist.tile([P, HP, N], f16)        # normalized OT stacked
            wp_sb = persist.tile([P, CT, C], f16)
            bpb_sb = persist.tile([P, C], f32)

            with tc.tile_pool(name="ph1", bufs=1) as ph1, \
                 tc.tile_pool(name="mix", bufs=2, space="PSUM") as mix, \
                 tc.tile_pool(name="et", bufs=32) as etp, \
                 tc.tile_pool(name="sm", bufs=1) as smp, \
                 tc.tile_pool(name="ys", bufs=NT) as ysp_pool, \
                 tc.tile_pool(name="ys2", bufs=6) as ys2_pool, \
                 tc.tile_pool(name="ps_s", bufs=2, space="PSUM") as ps_s, \
                 tc.tile_pool(name="ps_o", bufs=2, space="PSUM") as ps_o:
                x_sb = ph1.tile([P, CT, N], f16)
                id_sb = ph1.tile([P, P], f16)
                wq_sb = ph1.tile([P, HP, CT, 2 * D], f16)
                wk_sb = ph1.tile([P, HP, CT, 2 * D], f16)
                wv_sb = ph1.tile([P, CT, H * D], f16)
                # Input DMAs: x as three ~0.5MB transfers, FIRST on all
                # three queues (it gates every matmul; smaller chunks pay
                # ~2us fixed cost each and serialize), then the per-pair
                # q/k weight slices (each gates only its own pair's QK),
                # then v/p weights + bias behind.  This exact order measures
                # first-exp ~29us; every other tested shape (parity splits,
                # weights-first, 6-chunk interleaves) lands 32-37us.
                nc.sync.dma_start(x_sb[:, 0:2, :], xTr[:, 0:2, :])
                nc.scalar.dma_start(x_sb[:, 2:4, :], xTr[:, 2:4, :])
                nc.gpsimd.dma_start(x_sb[:, 4:6, :], xTr[:, 4:6, :])
                for j in range(HP):
                    nc.sync.dma_start(wq_sb[:, j], wqTr[:, j])
                    nc.scalar.dma_start(wk_sb[:, j], wkTr[:, j])
                nc.gpsimd.dma_start(id_sb[:], ident)
                nc.gpsimd.dma_start(wv_sb[:], wvTr[:])
                nc.gpsimd.dma_start(wp_sb[:], wpTr[:])
                nc.scalar.dma_start(bpb_sb[:], bpb)

                def at(us):
                    # sim-time floor: places instructions in the static
                    # per-engine order without affecting runtime waits
                    return tc.tile_wait_until(us / 1000.0)

                # scratch + exp-table preload + HAM warmup matmuls: keep the
                # PE clock warm across the input-DMA window (x lands ~13us).
                scr = ph1.tile([P, 512], f16)
                scrt = ph1.tile([P, 16], f32)
                nc.vector.memset(scr[:], 0.01)
                nc.vector.memset(vp[:, :, :, D:2 * D], 1.0)
                nc.scalar.activation(scrt[:], scr[:, 0:16], Exp, scale=1.0)
                wps = ps_o.tile([P, 512], f32, tag="o", name="wps")
                # one contiguous immediate block: floored bursts can
                # never be sim-placed inside the pre-x idle gap, but
                # immediates are guaranteed first — 16 back-to-back MMs run
                # to ~13.3us real, keeping the gap under the 3.4us HAM
                # window so QK(0) starts on a warm clock
                for i in range(16):
                    mm(wps[:], lhsT=scr[:, 0:128], rhs=scr[:],
                       start=True, stop=True)

                def emit_qk(j, cast=None):
                    for w_sb, dst in ((wq_sb, qt), (wk_sb, kt)):
                        for nh in range(2):
                            ps = mix.tile([P, 512], f32, tag="qk",
                                          name="qkps")
                            for c in range(CT):
                                mm(ps[:], lhsT=w_sb[:, j, c, :],
                                   rhs=x_sb[:, c, nh * 512:(nh + 1) * 512],
                                   start=(c == 0), stop=(c == CT - 1))
                            if cast is None:
                                nc.vector.tensor_copy(
                                    dst[:, j, nh * 512:(nh + 1) * 512],
                                    ps[:])
                            else:
                                cast(dst[:, j, nh * 512:(nh + 1) * 512],
                                     ps[:])

                emit_qk(0)

                def emit_v(t):
                    psa = mix.tile([P, 512], f32, tag="qk", name="psa")
                    psb = mix.tile([P, 512], f32, tag="qk", name="psb")
                    for c in range(CT):
                        lh = x_sb[:, c, t * P:(t + 1) * P]
                        mm(psa[:], lhsT=lh, rhs=wv_sb[:, c, 0:512],
                           start=(c == 0), stop=(c == CT - 1))
                        mm(psb[:, 0:256], lhsT=lh, rhs=wv_sb[:, c, 512:768],
                           start=(c == 0), stop=(c == CT - 1))
                    nc.vector.tensor_copy(
                        vp[:, t, 0:8, 0:D],
                        psa.rearrange("p (h d) -> p h d", d=D))
                    nc.vector.tensor_copy(
                        vp[:, t, 8:12, 0:D],
                        psb[:, 0:256].rearrange("p (h d) -> p h d", d=D))

                for t in range(NT):
                    emit_v(t)

                ets = {}

                def emit_scores_mt(j, mt):
                    # Both heads' scores for one nh-half share one PSUM
                    # tile, so each exp depends on both row-group matmuls
                    # and the scheduler cannot split the dual-stream pair.
                    s = {}
                    for nh in range(2):
                        s[nh] = ps_s.tile([P, 2, 512], f32, tag="s",
                                          name=f"s_{nh}")
                        ets[(j, mt, nh)] = etp.tile([P, 2, 512], f16,
                                                    tag="et", name=f"et_{nh}")
                    for nh in range(2):
                        for hh in range(2):   # adjacent => PE row-tiling
                            r0 = hh * D
                            mm(s[nh][:, hh, :],
                               lhsT=kt[r0:r0 + D, j, mt * P:(mt + 1) * P],
                               rhs=qt[r0:r0 + D, j, nh * 512:(nh + 1) * 512],
                               start=True, stop=True)
                    for nh in range(2):
                        nc.scalar.activation(ets[(j, mt, nh)][:], s[nh][:],
                                             Exp, scale=float(SCALE))

                def emit_pv_norm(j):
                    for hh in range(2):
                        h = 2 * j + hh
                        r0 = hh * D
                        pso = {nh: ps_o.tile([P, 512], f32, tag="o",
                                             name=f"o_{nh}")
                               for nh in range(2)}
                        for mt in range(NT):   # dense 16-MM PV burst
                            for nh in range(2):
                                mm(pso[nh][:],
                                   lhsT=vp[:, mt, h],
                                   rhs=ets[(j, mt, nh)][:, hh, :],
                                   start=(mt == 0), stop=(mt == NT - 1))
                        for nh in range(2):
                            sums = smp.tile([D, 512], f32, tag="sums")
                            rec = smp.tile([D, 512], f32, tag="rec")
                            nc.vector.tensor_copy(sums[:],
                                                  pso[nh][D:2 * D, :])
                            nc.vector.reciprocal_approx_fast(rec[:], sums[:])
                            nc.vector.tensor_mul(
                                ot[r0:r0 + D, j, nh * 512:(nh + 1) * 512],
                                pso[nh][0:D, :], rec[:])

                def emit_pv_norm_last(j):
                    # last pair: hh=0 accumulates in the early-freed ps_o
                    # slots (its MMs run as the last exps land); hh=1 goes
                    # through one [P,2,512] ps_s tile, whose slot frees at
                    # the last exp — so hh=1's PV overlaps hh=0's normalize.
                    # Z evacuation copies run on the now-idle ACT so DVE
                    # only does recip+mul.
                    pso0 = {nh: ps_o.tile([P, 512], f32, tag="o",
                                          name=f"ol0_{nh}")
                            for nh in range(2)}
                    for mt in range(NT):
                        for nh in range(2):
                            mm(pso0[nh][:],
                               lhsT=vp[:, mt, 2 * j],
                               rhs=ets[(j, mt, nh)][:, 0, :],
                               start=(mt == 0), stop=(mt == NT - 1))
                    pso1 = ps_s.tile([P, 2, 512], f32, tag="s", name="ol1")
                    for mt in range(NT):
                        for nh in range(2):
                            mm(pso1[:, nh, :],
                               lhsT=vp[:, mt, 2 * j + 1],
                               rhs=ets[(j, mt, nh)][:, 1, :],
                               start=(mt == 0), stop=(mt == NT - 1))
                    for nh in range(2):
                        sums = smp.tile([D, 512], f32, tag="sums")
                        rec = smp.tile([D, 512], f32, tag="rec")
                        nc.scalar.copy(sums[:], pso0[nh][D:2 * D, :])
                        nc.vector.reciprocal_approx_fast(rec[:], sums[:])
                        nc.vector.tensor_mul(
                            ot[0:D, j, nh * 512:(nh + 1) * 512],
                            pso0[nh][0:D, :], rec[:])
                    for nh in range(2):
                        sums1 = smp.tile([D, 512], f32, tag="sums")
                        rec1 = smp.tile([D, 512], f32, tag="rec")
                        nc.scalar.copy(sums1[:], pso1[D:2 * D, nh, :])
                        nc.vector.reciprocal_approx_fast(rec1[:], sums1[:])
                        nc.vector.tensor_mul(
                            ot[D:2 * D, j, nh * 512:(nh + 1) * 512],
                            pso1[0:D, nh, :], rec1[:])

                yre = y.rearrange("(t p) f -> t p f", p=P)
                ys = {}

                def emit_outproj_a(ts):
                    # pair 0-4 contributions of the output projection: PE
                    # filler for the ACT-bound last-pair exp window.  Bias
                    # is folded into the evacuation add.
                    for t in ts:
                        pa = mix.tile([P, 512], f32, tag="qk", name="ya")
                        pb = mix.tile([P, 512], f32, tag="qk", name="yb")
                        for c in range(HP - 1):
                            lh = ot[:, c, t * P:(t + 1) * P]
                            mm(pa[:], lhsT=lh, rhs=wp_sb[:, c, 0:512],
                               start=(c == 0), stop=(c == HP - 2))
                            mm(pb[:, 0:256], lhsT=lh,
                               rhs=wp_sb[:, c, 512:768],
                               start=(c == 0), stop=(c == HP - 2))
                        yt = ysp_pool.tile([P, C], f16, tag="ys")
                        ys[t] = yt
                        nc.vector.tensor_add(yt[:, 0:512], pa[:],
                                             bpb_sb[:, 0:512])
                        nc.vector.tensor_add(yt[:, 512:768], pb[:, 0:256],
                                             bpb_sb[:, 512:768])

                # software-pipelined: QK(j+1) is emitted BEFORE PV(j-1) so
                # its qt/kt evacuation copies land early in the DVE order.
                for j in range(HP):
                    for mt in range(NT):
                        emit_scores_mt(j, mt)
                        if mt == 1:
                            if j + 1 < HP:
                                emit_qk(j + 1)
                            if j > 0:
                                emit_pv_norm(j - 1)
                            if j == HP - 1:
                                emit_outproj_a(range(6))
                emit_outproj_a(range(6, NT))
                emit_pv_norm_last(HP - 1)

                # tail: pair-5 rank-128 update; the phase-A partial ys[t] is
                # folded in via an identity matmul so the evacuation is a
                # plain PSUM->SBUF copy, split across the now-idle ACT and
                # DVE engines.  Two half-width passes with ONE psum tile per
                # step: with pa+pb per t the 2-slot "qk" rotation degenerates
                # into a strict MM->copy->MM ladder (~2.3us/t); one tile per
                # step gives 2-deep pipelining (MM of t+1 under copy of t).
                for c0w, cw in ((0, 512), (512, 256)):
                    for t in range(NT):
                        # 4-deep psum rotation: mix + the ps_o slots freed
                        # by norm_last's hh=0 normalize
                        pool_b = mix if t % 2 == 0 else ps_o
                        tag_b = "qk" if t % 2 == 0 else "o"
                        pa = pool_b.tile([P, 512], f32, tag=tag_b,
                                         name="ya2")
                        lh = ot[:, HP - 1, t * P:(t + 1) * P]
                        mm(pa[:, 0:cw], lhsT=lh,
                           rhs=wp_sb[:, HP - 1, c0w:c0w + cw],
                           start=True, stop=False)
                        mm(pa[:, 0:cw], lhsT=id_sb[:],
                           rhs=ys[t][:, c0w:c0w + cw],
                           start=False, stop=True)
                        y2 = ys2_pool.tile([P, 512], f32, tag="ys2")
                        if t % 2 == 0:
                            nc.scalar.copy(y2[:, 0:cw], pa[:, 0:cw])
                        else:
                            nc.vector.tensor_copy(y2[:, 0:cw], pa[:, 0:cw])
                        # output DMA triggers stay off the scalar engine
                        # (a trigger costs ~0.6us ON the engine and would
                        # serialize with the ACT copies); the two halves go
                        # to different queues so the ~3MB drains in parallel
                        eng = nc.sync if c0w == 0 else nc.gpsimd
                        eng.dma_start(yre[t][:, c0w:c0w + cw], y2[:, 0:cw])

    nc.compile()
    return nc


def _get_nc():
    if "nc" not in _cache:
        _cache["nc"] = _build_nc()
    return _cache["nc"]


def _make_in_maps(x, Wq, Wk, Wv, Wp, bp):
    x = np.asarray(x, dtype=np.float32)
    # pair-major q/k weights: [HP, C, 2D] so each pair's slice is one
    # contiguous DMA
    wq = np.asarray(Wq, np.float32).reshape(HP, 2 * D, C)
    wk = np.asarray(Wk, np.float32).reshape(HP, 2 * D, C)
    wqT = np.ascontiguousarray(wq.transpose(0, 2, 1).astype(np.float16))
    wkT = np.ascontiguousarray(wk.transpose(0, 2, 1).astype(np.float16))
    wvT = np.ascontiguousarray(
        np.asarray(Wv, np.float32).reshape(H * D, C).T.astype(np.float16))
    wpT = np.ascontiguousarray(
        np.asarray(Wp, np.float32).T.astype(np.float16))
    bpb = np.ascontiguousarray(
        np.broadcast_to(np.asarray(bp, np.float32), (P, C)))
    ident = np.ascontiguousarray(np.eye(P, dtype=np.float16))
    in_maps = []
    for b in range(NCORES):
        in_maps.append({
            "xT": np.ascontiguousarray(x[b].T.astype(np.float16)),
            "wqT": wqT, "wkT": wkT, "wvT": wvT, "wpT": wpT, "bpb": bpb,
            "ident": ident,
        })
    return in_maps


def run(x, Wq, Wk, Wv, Wp, bp, trace=False):
    from concourse.bass_utils import run_bass_kernel_spmd
    nc = _get_nc()
    in_maps = _make_in_maps(x, Wq, Wk, Wv, Wp, bp)
    res = run_bass_kernel_spmd(nc, in_maps, list(range(NCORES)), trace=trace)
    out = np.stack([res.results[b]["y"] for b in range(NCORES)])
    return out, res


def kernel(x, Wq, Wk, Wv, Wp, bp):
    out, _ = run(x, Wq, Wk, Wv, Wp, bp)
    return out


# revision 76
# speedup vs baseline: 1.0008x; 1.0008x over previous
"""Trainium2 Bass kernel for nn_Attentionv2 (B=8, N=1024, C=768, H=12, D=64).

Strategy: data-parallel over batch — one batch element per NeuronCore (8 cores).
Per core, multi-head attention is computed entirely in the "transposed"
orientation so no on-chip transposes are needed:

  QT[h*64+d, n] = sum_c WqT[c, h*64+d] * xT[c, n]     (head-pair tiles)
  KT likewise; V[n, h*64+d] = sum_c xT[c, n-tile] * WvT[c, :]
  ST[m, n]  = sum_d KT[d, m] * QT[d, n]               (scores transposed;
               the two heads of a pair sit on partitions 0-63 / 64-127 so
               their K=64 matmuls row-tile into the two PE array halves)
  ET        = exp(ST * 1/8)                            (no max-subtraction:
                                                        scores are O(1) here)
  PV lhsT   = [V_h | ones(64 cols)]  =>  out rows 0-63 = OT_h (unnorm),
               rows 64-127 = softmax denominator replicated 64x (free
               partition-broadcast done by the PE)
  OT_norm   = OT * (1/Z)                               (reciprocal via DVE,
                                                        straight from PSUM)
  y[n, o]   = sum_c OT_norm[c, n] * WpT[c, o] + bp[o]

Matmul operands are fp16 (full-rate PE, fast weight loads, HAM-warm clocks);
all accumulation is fp32 in PSUM.

Changes vs the original baseline:
  - q/k weights are pair-major on the host so each pair's slice is one
    contiguous DMA; all x/w input DMAs are fully contiguous per partition
    via the contraction-row remap c = 6p + o (both MM operands of every
    contraction use the same map, so the accumulated sum is unchanged).
  - QK(j+1) is emitted BEFORE PV(j-1) at each pair's mt==1 slot so the
    qt/kt evacuation copies land early in the DVE static order and the
    next pair's first scores are not scheduled behind the PV+normalize
    block (the exp chain otherwise stalls ~3.5us at pair boundaries).
  - the output projection is split: the pair 0-4 contribution for 6 of 8
    row tiles runs as PE filler inside the ACT-bound last-pair exp window
    (reusing the idle "qk" PSUM slots) with the bias folded into that
    evacuation add; the tail then only needs the pair-5 rank-128 update.
  - the tail update folds the phase-A partial back in via an identity
    matmul, so the final evacuation is a plain PSUM->SBUF copy that
    alternates between the (by then idle) ACT and DVE engines.
  - the last pair's PV goes through the freed ps_s banks for hh=1 so it
    overlaps hh=0's normalize instead of serializing on the 2-slot ps_o
    rotation, and its Z-row evacuations run on ACT.

  - the tail runs as two half-width passes with ONE psum tile per step
    (two tiles per step turn the 2-slot rotation into a serial ladder),
    evacuation copies alternate ACT/DVE, and the output DMA is split
    across the sync and gpsimd queues (a DMA trigger costs ~0.6us ON the
    issuing engine and would serialize with the ACT copies).

NOTE on scheduling: the Tile scheduler's static per-engine order is very
sensitive to emission order and its cost model drifts from real DMA/HAM
timing; many locally-reasonable reorderings (x-first DMA shapes, V tiles
inside pair 0, sim-time floor pinning of the whole pipeline) measured
WORSE on hardware.  This configuration is the empirical optimum of ~20
variants: this binary measures 179.9-181.6us on a healthy device (vs
198-200us baseline; first exp 24.0us, PE busy 152.5us, 7 cold MMs);
the warmup is ONE block of 16 immediate filler MMs — floored bursts can
never be sim-placed inside the pre-x idle gap, immediates are guaranteed
first placement and keep the HAM gap under its 3.4us window; identical binaries vary ~3us run-to-run plus a slow
upward drift over a session, and the device intermittently enters a
~220-234us throttled state — never trust a single-run A/B.  Pool-size
changes (ets, ys2 depth) improved monotonically; emission reshuffles were
a lottery — prefer the former.

The two highest-leverage fixes, in order: (1) ets pool at 32 bufs = full
double-buffering of the exp->PV tiles; at 24 the next pair's scores
exhaust the pool before PV(j) frees it, stalling the exp chain 2.5-4us at
EVERY pair boundary (ets=32 moved exp-chain-end from 170.7 to 157.6us);
(2) the x-thirds-first DMA order above (first exp 29us vs 32-37).
"""

import numpy as np

P = 128
B, N, C = 8, 1024, 768
H, D = 12, 64
SCALE = D ** -0.5  # 0.125
CT = C // P   # 6 contraction chunks
NT = N // P   # 8 sequence tiles
HP = H // 2   # 6 head pairs
NCORES = 8

_cache = {}


def _build_nc():
    import concourse.bass as bass
    import concourse.mybir as mybir
    import concourse.tile as tile
    from concourse import bacc

    f32 = mybir.dt.float32
    f16 = mybir.dt.float16
    Exp = mybir.ActivationFunctionType.Exp

    nc = bacc.Bacc("TRN2", target_bir_lowering=False, debug=False,
                   enable_asserts=False)

    xT = nc.dram_tensor("xT", [C, N], f16, kind="ExternalInput").ap()
    ident = nc.dram_tensor("ident", [P, P], f16, kind="ExternalInput").ap()
    wqT = nc.dram_tensor("wqT", [HP, C, 2 * D], f16, kind="ExternalInput").ap()
    wkT = nc.dram_tensor("wkT", [HP, C, 2 * D], f16, kind="ExternalInput").ap()
    wvT = nc.dram_tensor("wvT", [C, H * D], f16, kind="ExternalInput").ap()
    wpT = nc.dram_tensor("wpT", [C, C], f16, kind="ExternalInput").ap()
    bpb = nc.dram_tensor("bpb", [P, C], f32, kind="ExternalInput").ap()
    y = nc.dram_tensor("y", [N, C], f32, kind="ExternalOutput").ap()

    mm = nc.tensor.matmul

    # contraction-row remap: partition p of chunk o holds row c = 6p + o,
    # so every per-partition DMA run is contiguous in HBM.  x/wq/wk/wv all
    # use this map (both matmul operands see the same permutation); wp keeps
    # chunk==pair / partition==row-within-pair since its contraction runs
    # over the concat-head dim whose layout is fixed by ot.
    xTr = xT.rearrange("(p o) n -> p o n", o=CT)
    wqTr = wqT.rearrange("h (p o) f -> p h o f", o=CT)
    wkTr = wkT.rearrange("h (p o) f -> p h o f", o=CT)
    wvTr = wvT.rearrange("(p o) f -> p o f", o=CT)
    wpTr = wpT.rearrange("(o p) f -> p o f", p=P)

    with tile.TileContext(nc) as tc:
        with tc.tile_pool(name="persist", bufs=1) as persist:
            qt = persist.tile([P, HP, N], f16)        # QT: head pair j rows
            kt = persist.tile([P, HP, N], f16)
            vp = persist.tile([P, NT, H, 2 * D], f16)  # [Vh | ones]
            ot = persist.tile([P, HP, N], f16)        # normalized OT stacked
            wp_sb = persist.tile([P, CT, C], f16)
            bpb_sb = persist.tile([P, C], f32)

            with tc.tile_pool(name="ph1", bufs=1) as ph1, \
                 tc.tile_pool(name="mix", bufs=2, space="PSUM") as mix, \
                 tc.tile_pool(name="et", bufs=32) as etp, \
                 tc.tile_pool(name="sm", bufs=1) as smp, \
                 tc.tile_pool(name="ys", bufs=NT) as ysp_pool, \
                 tc.tile_pool(name="ys2", bufs=6) as ys2_pool, \
                 tc.tile_pool(name="ps_s", bufs=2, space="PSUM") as ps_s, \
                 tc.tile_pool(name="ps_o", bufs=2, space="PSUM") as ps_o:
                x_sb = ph1.tile([P, CT, N], f16)
                id_sb = ph1.tile([P, P], f16)
                wq_sb = ph1.tile([P, HP, CT, 2 * D], f16)
                wk_sb = ph1.tile([P, HP, CT, 2 * D], f16)
                wv_sb = ph1.tile([P, CT, H * D], f16)
                # Input DMAs: x as three ~0.5MB transfers, FIRST on all
                # three queues (it gates every matmul; smaller chunks pay
                # ~2us fixed cost each and serialize), then the per-pair
                # q/k weight slices (each gates only its own pair's QK),
                # then v/p weights + bias behind.  This exact order measures
                # first-exp ~29us; every other tested shape (parity splits,
                # weights-first, 6-chunk interleaves) lands 32-37us.
                nc.sync.dma_start(x_sb[:, 0:2, :], xTr[:, 0:2, :])
                nc.scalar.dma_start(x_sb[:, 2:4, :], xTr[:, 2:4, :])
                nc.gpsimd.dma_start(x_sb[:, 4:6, :], xTr[:, 4:6, :])
                for j in range(HP):
                    nc.sync.dma_start(wq_sb[:, j], wqTr[:, j])
                    nc.scalar.dma_start(wk_sb[:, j], wkTr[:, j])
                nc.gpsimd.dma_start(id_sb[:], ident)
                nc.gpsimd.dma_start(wv_sb[:], wvTr[:])
                nc.gpsimd.dma_start(wp_sb[:], wpTr[:])
                nc.scalar.dma_start(bpb_sb[:], bpb)

                def at(us):
                    # sim-time floor: places instructions in the static
                    # per-engine order without affecting runtime waits
                    return tc.tile_wait_until(us / 1000.0)

                # scratch + exp-table preload + HAM warmup matmuls: keep the
                # PE clock warm across the input-DMA window (x lands ~13us).
                scr = ph1.tile([P, 512], f16)
                scrt = ph1.tile([P, 16], f32)
                nc.vector.memset(scr[:], 0.01)
                nc.vector.memset(vp[:, :, :, D:2 * D], 1.0)
                nc.scalar.activation(scrt[:], scr[:, 0:16], Exp, scale=1.0)
                wps = ps_o.tile([P, 512], f32, tag="o", name="wps")
                # one contiguous immediate block: floored bursts can
                # never be sim-placed inside the pre-x idle gap, but
                # immediates are guaranteed first — 16 back-to-back MMs run
                # to ~13.3us real, keeping the gap under the 3.4us HAM
                # window so QK(0) starts on a warm clock
                for i in range(16):
                    mm(wps[:], lhsT=scr[:, 0:128], rhs=scr[:],
                       start=True, stop=True)

                def emit_qk(j, cast=None):
                    for w_sb, dst in ((wq_sb, qt), (wk_sb, kt)):
                        for nh in range(2):
                            ps = mix.tile([P, 512], f32, tag="qk",
                                          name="qkps")
                            for c in range(CT):
                                mm(ps[:], lhsT=w_sb[:, j, c, :],
                                   rhs=x_sb[:, c, nh * 512:(nh + 1) * 512],
                                   start=(c == 0), stop=(c == CT - 1))
                            if cast is None:
                                nc.vector.tensor_copy(
                                    dst[:, j, nh * 512:(nh + 1) * 512],
                                    ps[:])
                            else:
                                cast(dst[:, j, nh * 512:(nh + 1) * 512],
                                     ps[:])

                emit_qk(0)

                def emit_v(t):
                    psa = mix.tile([P, 512], f32, tag="qk", name="psa")
                    psb = mix.tile([P, 512], f32, tag="qk", name="psb")
                    for c in range(CT):
                        lh = x_sb[:, c, t * P:(t + 1) * P]
                        mm(psa[:], lhsT=lh, rhs=wv_sb[:, c, 0:512],
                           start=(c == 0), stop=(c == CT - 1))
                        mm(psb[:, 0:256], lhsT=lh, rhs=wv_sb[:, c, 512:768],
                           start=(c == 0), stop=(c == CT - 1))
                    nc.vector.tensor_copy(
                        vp[:, t, 0:8, 0:D],
                        psa.rearrange("p (h d) -> p h d", d=D))
                    nc.vector.tensor_copy(
                        vp[:, t, 8:12, 0:D],
                        psb[:, 0:256].rearrange("p (h d) -> p h d", d=D))

                for t in range(NT):
                    emit_v(t)

                ets = {}

                def emit_scores_mt(j, mt):
                    # Both heads' scores for one nh-half share one PSUM
                    # tile, so each exp depends on both row-group matmuls
                    # and the scheduler cannot split the dual-stream pair.
                    s = {}
                    for nh in range(2):
                        s[nh] = ps_s.tile([P, 2, 512], f32, tag="s",
                                          name=f"s_{nh}")
                        ets[(j, mt, nh)] = etp.tile([P, 2, 512], f16,
                                                    tag="et", name=f"et_{nh}")
                    for nh in range(2):
                        for hh in range(2):   # adjacent => PE row-tiling
                            r0 = hh * D
                            mm(s[nh][:, hh, :],
                               lhsT=kt[r0:r0 + D, j, mt * P:(mt + 1) * P],
                               rhs=qt[r0:r0 + D, j, nh * 512:(nh + 1) * 512],
                               start=True, stop=True)
                    for nh in range(2):
                        nc.scalar.activation(ets[(j, mt, nh)][:], s[nh][:],
                                             Exp, scale=float(SCALE))

                def emit_pv_norm(j):
                    for hh in range(2):
                        h = 2 * j + hh
                        r0 = hh * D
                        pso = {nh: ps_o.tile([P, 512], f32, tag="o",
                                             name=f"o_{nh}")
                               for nh in range(2)}
                        for mt in range(NT):   # dense 16-MM PV burst
                            for nh in range(2):
                                mm(pso[nh][:],
                                   lhsT=vp[:, mt, h],
                                   rhs=ets[(j, mt, nh)][:, hh, :],
                                   start=(mt == 0), stop=(mt == NT - 1))
                        for nh in range(2):
                            sums = smp.tile([D, 512], f32, tag="sums")
                            rec = smp.tile([D, 512], f32, tag="rec")
                            nc.vector.tensor_copy(sums[:],
                                                  pso[nh][D:2 * D, :])
                            nc.vector.reciprocal_approx_fast(rec[:], sums[:])
                            nc.vector.tensor_mul(
                                ot[r0:r0 + D, j, nh * 512:(nh + 1) * 512],
                                pso[nh][0:D, :], rec[:])

                def emit_pv_norm_last(j):
                    # last pair: hh=0 accumulates in the early-freed ps_o
                    # slots (its MMs run as the last exps land); hh=1 goes
                    # through one [P,2,512] ps_s tile, whose slot frees at
                    # the last exp — so hh=1's PV overlaps hh=0's normalize.
                    # Z evacuation copies run on the now-idle ACT so DVE
                    # only does recip+mul.
                    pso0 = {nh: ps_o.tile([P, 512], f32, tag="o",
                                          name=f"ol0_{nh}")
                            for nh in range(2)}
                    for mt in range(NT):
                        for nh in range(2):
                            mm(pso0[nh][:],
                               lhsT=vp[:, mt, 2 * j],
                               rhs=ets[(j, mt, nh)][:, 0, :],
                               start=(mt == 0), stop=(mt == NT - 1))
                    pso1 = ps_s.tile([P, 2, 512], f32, tag="s", name="ol1")
                    for mt in range(NT):
                        for nh in range(2):
                            mm(pso1[:, nh, :],
                               lhsT=vp[:, mt, 2 * j + 1],
                               rhs=ets[(j, mt, nh)][:, 1, :],
                               start=(mt == 0), stop=(mt == NT - 1))
                    for nh in range(2):
                        sums = smp.tile([D, 512], f32, tag="sums")
                        rec = smp.tile([D, 512], f32, tag="rec")
                        nc.scalar.copy(sums[:], pso0[nh][D:2 * D, :])
                        nc.vector.reciprocal_approx_fast(rec[:], sums[:])
                        nc.vector.tensor_mul(
                            ot[0:D, j, nh * 512:(nh + 1) * 512],
                            pso0[nh][0:D, :], rec[:])
                    for nh in range(2):
                        sums1 = smp.tile([D, 512], f32, tag="sums")
                        rec1 = smp.tile([D, 512], f32, tag="rec")
                        nc.scalar.copy(sums1[:], pso1[D:2 * D, nh, :])
                        nc.vector.reciprocal_approx_fast(rec1[:], sums1[:])
                        nc.vector.tensor_mul(
                            ot[D:2 * D, j, nh * 512:(nh + 1) * 512],
                            pso1[0:D, nh, :], rec1[:])

                yre = y.rearrange("(t p) f -> t p f", p=P)
                ys = {}

                def emit_outproj_a(ts):
                    # pair 0-4 contributions of the output projection: PE
                    # filler for the ACT-bound last-pair exp window.  Bias
                    # is folded into the evacuation add.
                    for t in ts:
                        pa = mix.tile([P, 512], f32, tag="qk", name="ya")
                        pb = mix.tile([P, 512], f32, tag="qk", name="yb")
                        for c in range(HP - 1):
                            lh = ot[:, c, t * P:(t + 1) * P]
                            mm(pa[:], lhsT=lh, rhs=wp_sb[:, c, 0:512],
                               start=(c == 0), stop=(c == HP - 2))
                            mm(pb[:, 0:256], lhsT=lh,
                               rhs=wp_sb[:, c, 512:768],
                               start=(c == 0), stop=(c == HP - 2))
                        yt = ysp_pool.tile([P, C], f16, tag="ys")
                        ys[t] = yt
                        nc.vector.tensor_add(yt[:, 0:512], pa[:],
                                             bpb_sb[:, 0:512])
                        nc.vector.tensor_add(yt[:, 512:768], pb[:, 0:256],
                                             bpb_sb[:, 512:768])

                # software-pipelined: QK(j+1) is emitted BEFORE PV(j-1) so
                # its qt/kt evacuation copies land early in the DVE order.
                for j in range(HP):
                    for mt in range(NT):
                        emit_scores_mt(j, mt)
                        if mt == 1:
                            if j + 1 < HP:
                                emit_qk(j + 1)
                            if j > 0:
                                emit_pv_norm(j - 1)
                            if j == HP - 1:
                                emit_outproj_a(range(6))
                emit_outproj_a(range(6, NT))
                emit_pv_norm_last(HP - 1)

                # tail: pair-5 rank-128 update; the phase-A partial ys[t] is
                # folded in via an identity matmul so the evacuation is a
                # plain PSUM->SBUF copy, split across the now-idle ACT and
                # DVE engines.  Two half-width passes with ONE psum tile per
                # step: with pa+pb per t the 2-slot "qk" rotation degenerates
                # into a strict MM->copy->MM ladder (~2.3us/t); one tile per
                # step gives 2-deep pipelining (MM of t+1 under copy of t).
                for c0w, cw in ((0, 512), (512, 256)):
                    for t in range(NT):
                        # 4-deep psum rotation: mix + the ps_o slots freed
                        # by norm_last's hh=0 normalize
                        pool_b = mix if t % 2 == 0 else ps_o
                        tag_b = "qk" if t % 2 == 0 else "o"
                        pa = pool_b.tile([P, 512], f32, tag=tag_b,
                                         name="ya2")
                        lh = ot[:, HP - 1, t * P:(t + 1) * P]
                        mm(pa[:, 0:cw], lhsT=lh,
                           rhs=wp_sb[:, HP - 1, c0w:c0w + cw],
                           start=True, stop=False)
                        mm(pa[:, 0:cw], lhsT=id_sb[:],
                           rhs=ys[t][:, c0w:c0w + cw],
                           start=False, stop=True)
                        y2 = ys2_pool.tile([P, 512], f32, tag="ys2")
                        if t % 2 == 0:
                            nc.scalar.copy(y2[:, 0:cw], pa[:, 0:cw])
                        else:
                            nc.vector.tensor_copy(y2[:, 0:cw], pa[:, 0:cw])
                        # output DMA triggers stay off the scalar engine
                        # (a trigger costs ~0.6us ON the engine and would
                        # serialize with the ACT copies); the two halves go
                        # to different queues so the ~3MB drains in parallel
                        eng = nc.sync if c0w == 0 else nc.gpsimd
                        eng.dma_start(yre[t][:, c0w:c0w + cw], y2[:, 0:cw])

    nc.compile()
    return nc


def _get_nc():
    if "nc" not in _cache:
        _cache["nc"] = _build_nc()
    return _cache["nc"]


def _make_in_maps(x, Wq, Wk, Wv, Wp, bp):
    x = np.asarray(x, dtype=np.float32)
    # pair-major q/k weights: [HP, C, 2D] so each pair's slice is one
    # contiguous DMA
    wq = np.asarray(Wq, np.float32).reshape(HP, 2 * D, C)
    wk = np.asarray(Wk, np.float32).reshape(HP, 2 * D, C)
    wqT = np.ascontiguousarray(wq.transpose(0, 2, 1).astype(np.float16))
    wkT = np.ascontiguousarray(wk.transpose(0, 2, 1).astype(np.float16))
    wvT = np.ascontiguousarray(
        np.asarray(Wv, np.float32).reshape(H * D, C).T.astype(np.float16))
    wpT = np.ascontiguousarray(
        np.asarray(Wp, np.float32).T.astype(np.float16))
    bpb = np.ascontiguousarray(
        np.broadcast_to(np.asarray(bp, np.float32), (P, C)))
    ident = np.ascontiguousarray(np.eye(P, dtype=np.float16))
    in_maps = []
    for b in range(NCORES):
        in_maps.append({
            "xT": np.ascontiguousarray(x[b].T.astype(np.float16)),
            "wqT": wqT, "wkT": wkT, "wvT": wvT, "wpT": wpT, "bpb": bpb,
            "ident": ident,
        })
    return in_maps


def run(x, Wq, Wk, Wv, Wp, bp, trace=False):
    from concourse.bass_utils import run_bass_kernel_spmd
    nc = _get_nc()
    in_maps = _make_in_maps(x, Wq, Wk, Wv, Wp, bp)
    res = run_bass_kernel_spmd(nc, in_maps, list(range(NCORES)), trace=trace)
    out = np.stack([res.results[b]["y"] for b in range(NCORES)])
    return out, res


def kernel(x, Wq, Wk, Wv, Wp, bp):
    out, _ = run(x, Wq, Wk, Wv, Wp, bp)
    return out
